# revision 1
# baseline (speedup 1.0000x reference)
"""Bass/Trainium2 kernel for nn_LocalAggregator (GNN message passing).

Math per batch b (hidden [64,128], adj [64,64] in {0..4}, a [4,128]):
    e_k[i,j] = leakyrelu_{0.2}( sum_d hidden[i,d]*hidden[j,d]*a[k,d] )
    alpha    = softmax_j( where(adj==k+1, e_k, -9e15) )
    out      = alpha @ hidden

Device strategy (8 cores, pure batch data-parallel, 64 batches/core,
processed in "quads" of 4 batches):
  - e_k is SYMMETRIC in (i,j).  We exploit this: the PSUM tile holding
    e_k[i,j] can be reinterpreted as e_k[j,i], so masking it with the
    host-TRANSPOSED adjacency produces the transposed attention weights
    w^T[j,i] directly -- no on-chip transposes anywhere.
  - leaky-relu runs on the ACT engine as Prelu(alpha=0.2) while it
    evacuates PSUM; Exp follows as a second ACT pass.
  - Selection is a multiplicative one-hot: w = (adjT==k+1) * exp(...).
    Masked entries become exactly 0, matching exp(-9e15 - max) == 0.
  - A ones-column appended to hidden makes the final matmul emit the
    softmax denominator s_i alongside alpha@h; normalize by 1/s_i after.
  - Host pre-packs bf16 layouts; matmuls in bf16 (fp32 PSUM accumulate).
"""

import numpy as np
import ml_dtypes

from contextlib import ExitStack

import concourse.bass as bass
import concourse.tile as tile
from concourse import bacc, mybir
from concourse._compat import with_exitstack
from concourse.bass_utils import run_bass_kernel_spmd

BF16 = mybir.dt.bfloat16
F32 = mybir.dt.float32
ALU = mybir.AluOpType
ACTF = mybir.ActivationFunctionType

B, N, D, K = 512, 64, 128, 4
NCORES = 8
BPC = B // NCORES          # 64 batches per core
QUADS = BPC // 4           # 16 quads of 4 batches per core
HHW = 132                  # hidden cols + ones col + pad (128 data, 1 ones, 3 zero)


@with_exitstack
def _kernel_body(ctx, tc, hT_d, hh_d, adjT_d, aT_d, out_d):
    nc = tc.nc

    const_pool = ctx.enter_context(tc.tile_pool(name="const", bufs=1))
    in_pool = ctx.enter_context(tc.tile_pool(name="inp", bufs=3))
    work_pool = ctx.enter_context(tc.tile_pool(name="work", bufs=3))
    psum_pool = ctx.enter_context(tc.tile_pool(name="psum", bufs=2, space="PSUM"))
    opsum_pool = ctx.enter_context(tc.tile_pool(name="opsum", bufs=2, space="PSUM"))
    out_pool = ctx.enter_context(tc.tile_pool(name="outp", bufs=3))

    # --- one-time constants ---
    a_sb = const_pool.tile([128, 4], F32)          # a^T : [d, k]
    nc.sync.dma_start(out=a_sb[:], in_=aT_d[:, :])
    # kpat[:, p*256 + k*64 + c] = k+1  (compare target for the one-hot)
    kpat = const_pool.tile([128, 512], BF16)
    for p in range(2):
        for k in range(K):
            nc.gpsimd.memset(kpat[:, p * 256 + k * 64 : p * 256 + (k + 1) * 64],
                             float(k + 1))

    for q in range(QUADS):
        # ---- loads ----
        # hT [128=d, 256=(l,i)] for the 4 batches l=0..3 of this quad
        hT = in_pool.tile([128, 256], BF16, tag="hT")
        nc.sync.dma_start(out=hT[:], in_=hT_d[q])
        # adjT [128=(u,r), 128=(p,c)] = adj[4q+2p+u][c, r]
        adjT = in_pool.tile([128, 128], BF16, tag="adjT")
        nc.sync.dma_start(out=adjT[:], in_=adjT_d[q])
        # hh[p] [128=(u,j), 132] original-layout hidden rows + ones col
        hh = []
        for p in range(2):
            t = in_pool.tile([128, HHW], BF16, tag=f"hh{p}")
            nc.sync.dma_start(
                out=t[:],
                in_=hh_d[4 * q + 2 * p : 4 * q + 2 * p + 2].flatten_outer_dims(),
            )
            hh.append(t)

        # ---- w_all[d, (l,k,j)] = hT[d, (l,j)] * a[k,d] ----
        # 4 per-k tensor_scalar ops on the (otherwise idle) Pool engine.
        w_all = work_pool.tile([128, 1024], BF16, tag="w_all")
        hTv = hT[:].rearrange("p (l j) -> p l j", l=4)
        w_allv = w_all[:].rearrange("p (l k j) -> p l k j", l=4, k=4)
        for k in range(K):
            nc.gpsimd.tensor_scalar(
                w_allv[:, :, k, :], hTv, a_sb[:, k : k + 1], None, ALU.mult)

        # ---- e4[(u,i), (p,k,j)] = e_k^{l=2p+u}[i,j] : 4 matmuls, K=d=128 ----
        e4 = psum_pool.tile([128, 512], F32, tag="e4")
        for l in range(4):
            p, u = l // 2, l % 2
            nc.tensor.matmul(
                e4[u * 64 : (u + 1) * 64, p * 256 : (p + 1) * 256],
                lhsT=hT[:, l * 64 : (l + 1) * 64],
                rhs=w_all[:, l * 256 : (l + 1) * 256],
                start=True, stop=True,
                tile_position=(0, u * 64),
            )

        # ---- xm = exp(leakyrelu(e)) : Prelu evacuates PSUM, then Exp ----
        lr4 = work_pool.tile([128, 512], F32, tag="lr4")
        nc.scalar.activation(lr4[:], e4[:], ACTF.Prelu, alpha=0.2)
        xm = work_pool.tile([128, 512], BF16, tag="xm")
        nc.scalar.activation(xm[:], lr4[:], ACTF.Exp)

        # ---- one-hot select via transposed adj (symmetry trick) ----
        ind = work_pool.tile([128, 512], BF16, tag="ind")
        adjv = (adjT[:].rearrange("p (t c) -> p t c", t=2)
                .unsqueeze(2).broadcast_to([128, 2, 4, 64]))
        kv = kpat[:].rearrange("p (t k c) -> p t k c", t=2, k=4)
        nc.vector.tensor_tensor(
            ind[:].rearrange("p (t k c) -> p t k c", t=2, k=4),
            adjv, kv, ALU.is_equal)
        w4 = work_pool.tile([128, 512], BF16, tag="w4")
        nc.vector.tensor_mul(w4[:], xm[:], ind[:])

        # ---- sum over k: w_sumT[(u,j), (p,i)] ----
        w4v = w4[:].rearrange("p (t k c) -> p t k c", t=2, k=4)
        t2 = work_pool.tile([128, 256], BF16, tag="t2")
        t2v = t2[:].rearrange("p (t k c) -> p t k c", t=2, k=2)
        nc.vector.tensor_tensor(t2v, w4v[:, :, 0:2, :], w4v[:, :, 2:4, :], ALU.add)
        wsum = work_pool.tile([128, 128], BF16, tag="wsum")
        wsv = wsum[:].rearrange("p (t c) -> p t c", t=2)
        nc.vector.tensor_tensor(wsv, t2v[:, :, 0, :], t2v[:, :, 1, :], ALU.add)

        # ---- out_p[(u,i), 0:128] = sum_j w^T[j,i] h[j,d]; col 128 = denom ----
        ops = []
        for p in range(2):
            t = opsum_pool.tile([128, HHW], F32, tag=f"ops{p}")
            ops.append(t)
        for l in range(4):
            p, u = l // 2, l % 2
            nc.tensor.matmul(
                ops[p][u * 64 : (u + 1) * 64, :],
                lhsT=wsum[u * 64 : (u + 1) * 64, p * 64 : (p + 1) * 64],
                rhs=hh[p][u * 64 : (u + 1) * 64, :],
                start=True, stop=True,
                tile_position=(u * 64, u * 64),
            )

        # ---- normalize rows by 1/denominator and store ----
        # (one scale on DVE, one on ACT to balance engine load)
        for p in range(2):
            r = work_pool.tile([128, 1], F32, tag=f"r{p}")
            nc.vector.reciprocal(r[:], ops[p][:, 128:129])
            osb = out_pool.tile([128, 128], F32, tag=f"osb{p}")
            if p == 0:
                nc.vector.tensor_scalar(osb[:], ops[p][:, 0:128], r[:], None, ALU.mult)
            else:
                nc.scalar.activation(osb[:], ops[p][:, 0:128], ACTF.Copy,
                                     scale=r[:])
            nc.sync.dma_start(
                out=out_d[4 * q + 2 * p : 4 * q + 2 * p + 2].flatten_outer_dims(),
                in_=osb[:],
            )


def build_nc():
    nc = bacc.Bacc("TRN2", target_bir_lowering=False, debug=False)
    hT_d = nc.dram_tensor("ht", [QUADS, 128, 256], BF16, kind="ExternalInput").ap()
    hh_d = nc.dram_tensor("hh", [BPC, 64, HHW], BF16, kind="ExternalInput").ap()
    adjT_d = nc.dram_tensor("adjt", [QUADS, 128, 128], BF16, kind="ExternalInput").ap()
    aT_d = nc.dram_tensor("at", [128, 4], F32, kind="ExternalInput").ap()
    out_d = nc.dram_tensor("out", [BPC, 64, 128], F32, kind="ExternalOutput").ap()
    with tile.TileContext(nc) as tc:
        _kernel_body(tc, hT_d, hh_d, adjT_d, aT_d, out_d)
    nc.compile()
    return nc


def prep_inputs(hidden, adj, a):
    """Host-side packing: bf16 casts, transposed/interleaved layouts, shards."""
    bf = ml_dtypes.bfloat16
    hidden = np.asarray(hidden, dtype=np.float32)
    adj = np.asarray(adj)
    a = np.asarray(a, dtype=np.float32)

    hb = hidden.astype(bf)                                   # [B, 64, 128]
    hh = np.zeros((B, N, HHW), dtype=bf)
    hh[:, :, 0:D] = hb
    hh[:, :, D] = bf(1.0)

    # hT_q[q, d, l*64+i] = hidden[4q+l, i, d]
    hT = (hb.transpose(0, 2, 1)                              # [B, d, i]
          .reshape(B // 4, 4, D, N)                          # [q, l, d, i]
          .transpose(0, 2, 1, 3)                             # [q, d, l, i]
          .reshape(B // 4, D, 4 * N))
    hT = np.ascontiguousarray(hT)

    # adjT_q[q, u*64+r, p*64+c] = adj[4q+2p+u][c, r]
    adjT = adj.transpose(0, 2, 1).astype(bf)                 # [b, r, c]
    adjTq = (adjT.reshape(B // 4, 2, 2, N, N)                # [q, p, u, r, c]
             .transpose(0, 2, 3, 1, 4)                       # [q, u, r, p, c]
             .reshape(B // 4, 2 * N, 2 * N))
    adjTq = np.ascontiguousarray(adjTq)

    aT = np.ascontiguousarray(a.T).astype(np.float32)        # [128, 4]

    in_maps = []
    for c in range(NCORES):
        bsl = slice(c * BPC, (c + 1) * BPC)
        qsl = slice(c * QUADS, (c + 1) * QUADS)
        in_maps.append({
            "ht": np.ascontiguousarray(hT[qsl]),
            "hh": np.ascontiguousarray(hh[bsl]),
            "adjt": np.ascontiguousarray(adjTq[qsl]),
            "at": aT,
        })
    return in_maps


_NC_CACHE = {}


def run_device(hidden, adj, a, **spmd_kwargs):
    if "nc" not in _NC_CACHE:
        _NC_CACHE["nc"] = build_nc()
    nc = _NC_CACHE["nc"]
    in_maps = prep_inputs(hidden, adj, a)
    res = run_bass_kernel_spmd(nc, in_maps, list(range(NCORES)), **spmd_kwargs)
    out = np.concatenate([res.results[c]["out"] for c in range(NCORES)], axis=0)
    return out.reshape(B, N, D).astype(np.float32), res


def kernel(hidden, adj, a):
    out, _ = run_device(hidden, adj, a)
    return out



# revision 5
# speedup vs baseline: 4.9182x; 4.9182x over previous
"""Bass/Trainium2 kernel for nn_LocalAggregator (GNN message passing).

Math per batch b (hidden [64,128], adj [64,64] in {0..4}, a [4,128]):
    e_k[i,j] = leakyrelu_{0.2}( sum_d hidden[i,d]*hidden[j,d]*a[k,d] )
    alpha    = softmax_j( where(adj==k+1, e_k, -9e15) )
    out      = alpha @ hidden

Device strategy (8 cores, pure batch data-parallel, 64 batches/core,
processed in "octs" of 8 batches):
  - e_k is SYMMETRIC in (i,j): the PSUM tile holding e_k[i,j] doubles as
    e_k[j,i], so masking with the host-TRANSPOSED adjacency yields the
    transposed attention weights directly -- no on-chip transposes.
  - w_all[d,(l,k,j)] = h[j,d]*a[k,d] is ONE DVE tensor_tensor with
    broadcast reads (hT repeated over k, aPat repeated over l).
  - leaky-relu runs on ACT as Prelu(0.2) evacuating PSUM; Exp follows.
  - Selection is a multiplicative one-hot (is_equal vs a constant k
    pattern); masked entries become exactly 0.
  - k-sum adds run on GpSimd to relieve the vector engine.
  - A ones-column appended to hidden makes the output matmul emit the
    softmax denominator; the division happens on the HOST (the raw
    numerator+denominator PSUM tile is DMA'd out directly).
"""

import numpy as np
import ml_dtypes

from contextlib import ExitStack

import concourse.bass as bass
import concourse.tile as tile
from concourse import bacc, mybir
from concourse._compat import with_exitstack
from concourse.bass_utils import run_bass_kernel_spmd

BF16 = mybir.dt.bfloat16
F32 = mybir.dt.float32
ALU = mybir.AluOpType
ACTF = mybir.ActivationFunctionType

B, N, D, K = 512, 64, 128, 4
NCORES = 8
BPC = B // NCORES          # 64 batches per core
OCTS = BPC // 8            # 8 octs of 8 batches per core
HHW = 132                  # hidden cols + ones col + pad (128 data, 1 ones, 3 zero)
OW = 4 * HHW               # output tile cols (4 pair-blocks of 132)


@with_exitstack
def _kernel_body(ctx, tc, hT_d, hh_d, adjT_d, aPat_d, out_d):
    nc = tc.nc

    const_pool = ctx.enter_context(tc.tile_pool(name="const", bufs=1))
    in_pool = ctx.enter_context(tc.tile_pool(name="inp", bufs=3))
    work_pool = ctx.enter_context(tc.tile_pool(name="work", bufs=2))
    psum_pool = ctx.enter_context(tc.tile_pool(name="psum", bufs=2, space="PSUM"))
    opsum_pool = ctx.enter_context(tc.tile_pool(name="opsum", bufs=2, space="PSUM"))

    # --- one-time constants ---
    # aPat[d, k*64+j] = a[k,d]  (host-precomputed)
    aPat = const_pool.tile([128, 256], BF16)
    nc.sync.dma_start(out=aPat[:], in_=aPat_d[:, :])
    # kpat[:, c*256 + k*64 + s] = k+1  (compare target for the one-hot)
    kpat = const_pool.tile([128, 1024], BF16)
    for c in range(4):
        for k in range(K):
            nc.gpsimd.memset(kpat[:, c * 256 + k * 64 : c * 256 + (k + 1) * 64],
                             float(k + 1))

    for q in range(OCTS):
        # ---- loads ----
        # hT [128=d, 512=(l,i)] for the 8 batches l=0..7 of this oct
        hT = in_pool.tile([128, 512], BF16, tag="hT")
        nc.sync.dma_start(out=hT[:], in_=hT_d[q])
        # adjT [128=(u,r), 256=(c,s)] = adj[oct, 2c+u][s, r]
        adjT = in_pool.tile([128, 256], BF16, tag="adjT")
        nc.sync.dma_start(out=adjT[:], in_=adjT_d[q])
        # hh [128=(u,j), 528=(c, d+ones+pad)] original-layout hidden rows
        hh = in_pool.tile([128, OW], BF16, tag="hh")
        nc.sync.dma_start(out=hh[:], in_=hh_d[q])

        # ---- w_all[d, (l,k,j)] = hT[d, (l,j)] * a[k,d] : one DVE op ----
        w_all = work_pool.tile([128, 2048], BF16, tag="w_all")
        w_allv = w_all[:].rearrange("p (l k j) -> p l k j", l=8, k=4)
        hTv = (hT[:].rearrange("p (l j) -> p l j", l=8)
               .unsqueeze(2).broadcast_to([128, 8, 4, 64]))
        aPatv = (aPat[:].rearrange("p (k j) -> p k j", k=4)
                 .unsqueeze(1).broadcast_to([128, 8, 4, 64]))
        nc.vector.tensor_tensor(w_allv, hTv, aPatv, ALU.mult)

        # ---- e4[(u,i), (c,k,j)] = e_k^{l=2c+u}[i,j] : 8 matmuls ----
        e4 = psum_pool.tile([128, 1024], F32, tag="e4")
        for l in range(8):
            c, u = l // 2, l % 2
            nc.tensor.matmul(
                e4[u * 64 : (u + 1) * 64, c * 256 : (c + 1) * 256],
                lhsT=hT[:, l * 64 : (l + 1) * 64],
                rhs=w_all[:, l * 256 : (l + 1) * 256],
                start=True, stop=True,
                tile_position=(0, u * 64),
            )

        # ---- xm = exp(leakyrelu(e)) : Prelu evacuates PSUM, then Exp ----
        lr4 = work_pool.tile([128, 1024], F32, tag="lr4")
        nc.scalar.activation(lr4[:], e4[:], ACTF.Prelu, alpha=0.2)
        xm = work_pool.tile([128, 1024], BF16, tag="xm")
        nc.scalar.activation(xm[:], lr4[:], ACTF.Exp)

        # ---- one-hot select via transposed adj (symmetry trick) ----
        ind = work_pool.tile([128, 1024], BF16, tag="ind")
        adjv = (adjT[:].rearrange("p (c s) -> p c s", c=4)
                .unsqueeze(2).broadcast_to([128, 4, 4, 64]))
        kv = kpat[:].rearrange("p (c k s) -> p c k s", c=4, k=4)
        nc.vector.tensor_tensor(
            ind[:].rearrange("p (c k s) -> p c k s", c=4, k=4),
            adjv, kv, ALU.is_equal)
        w4 = work_pool.tile([128, 1024], BF16, tag="w4")
        nc.vector.tensor_mul(w4[:], xm[:], ind[:])

        # ---- sum over k (GpSimd): wsum[(u,j), (c,i)] ----
        w4v = w4[:].rearrange("p (c k s) -> p c k s", c=4, k=4)
        t2 = work_pool.tile([128, 512], BF16, tag="t2")
        t2v = t2[:].rearrange("p (c k s) -> p c k s", c=4, k=2)
        nc.gpsimd.tensor_tensor(t2v, w4v[:, :, 0:2, :], w4v[:, :, 2:4, :], ALU.add)
        wsum = work_pool.tile([128, 256], BF16, tag="wsum")
        wsv = wsum[:].rearrange("p (c s) -> p c s", c=4)
        nc.gpsimd.tensor_tensor(wsv, t2v[:, :, 0, :], t2v[:, :, 1, :], ALU.add)

        # ---- out[(u,i), (c,:)] = sum_j w^T[j,i] hh[j,:]; col 128 = denom ----
        # (two PSUM tiles: a [128, 4*132] f32 tile would cross the 2 KiB
        #  bank boundary mid-matmul at the c=3 block)
        ops0 = opsum_pool.tile([128, 2 * HHW], F32, tag="ops0")
        ops1 = opsum_pool.tile([128, 2 * HHW], F32, tag="ops1")
        ops = [ops0, ops1]
        for l in range(8):
            c, u = l // 2, l % 2
            nc.tensor.matmul(
                ops[c // 2][u * 64 : (u + 1) * 64,
                            (c % 2) * HHW : (c % 2 + 1) * HHW],
                lhsT=wsum[u * 64 : (u + 1) * 64, c * 64 : (c + 1) * 64],
                rhs=hh[u * 64 : (u + 1) * 64, c * HHW : (c + 1) * HHW],
                start=True, stop=True,
                tile_position=(u * 64, u * 64),
            )

        # ---- raw numerator+denominator to HBM (host divides) ----
        osb = work_pool.tile([128, OW], F32, tag="osb")
        for h in range(2):
            nc.scalar.activation(osb[:, h * 2 * HHW : (h + 1) * 2 * HHW],
                                 ops[h][:], ACTF.Copy)
        nc.sync.dma_start(out=out_d[q], in_=osb[:])


def build_nc():
    nc = bacc.Bacc("TRN2", target_bir_lowering=False, debug=False)
    hT_d = nc.dram_tensor("ht", [OCTS, 128, 512], BF16, kind="ExternalInput").ap()
    hh_d = nc.dram_tensor("hh", [OCTS, 128, OW], BF16, kind="ExternalInput").ap()
    adjT_d = nc.dram_tensor("adjt", [OCTS, 128, 256], BF16, kind="ExternalInput").ap()
    aPat_d = nc.dram_tensor("apat", [128, 256], BF16, kind="ExternalInput").ap()
    out_d = nc.dram_tensor("out", [OCTS, 128, OW], F32, kind="ExternalOutput").ap()
    with tile.TileContext(nc) as tc:
        _kernel_body(tc, hT_d, hh_d, adjT_d, aPat_d, out_d)
    nc.compile()
    return nc


def prep_inputs(hidden, adj, a):
    """Host-side packing: bf16 casts, transposed/interleaved layouts, shards."""
    bf = ml_dtypes.bfloat16
    hidden = np.asarray(hidden, dtype=np.float32)
    adj = np.asarray(adj)
    a = np.asarray(a, dtype=np.float32)

    hb = hidden.astype(bf)                                   # [B, 64, 128]

    # hh_oct[q][u*64+j, c*132+d] = hidden[8q+2c+u, j, d]; col 128 = 1
    hh = np.zeros((B, N, HHW), dtype=bf)
    hh[:, :, 0:D] = hb
    hh[:, :, D] = bf(1.0)
    hhq = (hh.reshape(B // 8, 4, 2, N, HHW)                  # [q, c, u, j, :]
           .transpose(0, 2, 3, 1, 4)                         # [q, u, j, c, :]
           .reshape(B // 8, 2 * N, 4 * HHW))
    hhq = np.ascontiguousarray(hhq)

    # hT_q[q, d, l*64+i] = hidden[8q+l, i, d]
    hT = (hb.transpose(0, 2, 1)                              # [B, d, i]
          .reshape(B // 8, 8, D, N)                          # [q, l, d, i]
          .transpose(0, 2, 1, 3)                             # [q, d, l, i]
          .reshape(B // 8, D, 8 * N))
    hT = np.ascontiguousarray(hT)

    # adjT_q[q, u*64+r, c*64+s] = adj[8q+2c+u][s, r]
    adjT = adj.transpose(0, 2, 1).astype(bf)                 # [b, r, s]
    adjTq = (adjT.reshape(B // 8, 4, 2, N, N)                # [q, c, u, r, s]
             .transpose(0, 2, 3, 1, 4)                       # [q, u, r, c, s]
             .reshape(B // 8, 2 * N, 4 * N))
    adjTq = np.ascontiguousarray(adjTq)

    # aPat[d, k*64+j] = a[k, d]
    aPat = np.ascontiguousarray(
        np.broadcast_to(a.T[:, :, None], (D, K, N)).reshape(D, K * N)
    ).astype(bf)

    in_maps = []
    for cidx in range(NCORES):
        qsl = slice(cidx * OCTS, (cidx + 1) * OCTS)
        in_maps.append({
            "ht": np.ascontiguousarray(hT[qsl]),
            "hh": np.ascontiguousarray(hhq[qsl]),
            "adjt": np.ascontiguousarray(adjTq[qsl]),
            "apat": aPat,
        })
    return in_maps


_NC_CACHE = {}


def run_device(hidden, adj, a, **spmd_kwargs):
    if "nc" not in _NC_CACHE:
        _NC_CACHE["nc"] = build_nc()
    nc = _NC_CACHE["nc"]
    in_maps = prep_inputs(hidden, adj, a)
    res = run_bass_kernel_spmd(nc, in_maps, list(range(NCORES)), **spmd_kwargs)
    raw = np.stack([res.results[c]["out"] for c in range(NCORES)], axis=0)
    # raw: [NCORES, OCTS, 128, 528] f32 -> [b, i, 132] -> normalize
    o = (raw.reshape(NCORES * OCTS, 2, N, 4, HHW)            # [q, u, i, c, :]
         .transpose(0, 3, 1, 2, 4)                           # [q, c, u, i, :]
         .reshape(B, N, HHW))
    out = o[:, :, 0:D] / o[:, :, D:D + 1]
    return np.ascontiguousarray(out, dtype=np.float32), res


def kernel(hidden, adj, a):
    out, _ = run_device(hidden, adj, a)
    return out


# revision 6
# speedup vs baseline: 5.1349x; 1.0441x over previous
"""Bass/Trainium2 kernel for nn_LocalAggregator (GNN message passing).

Math per batch b (hidden [64,128], adj [64,64] in {0..4}, a [4,128]):
    e_k[i,j] = leakyrelu_{0.2}( sum_d hidden[i,d]*hidden[j,d]*a[k,d] )
    alpha    = softmax_j( where(adj==k+1, e_k, -9e15) )
    out      = alpha @ hidden

Device strategy (8 cores, pure batch data-parallel, 64 batches/core,
processed in "octs" of 8 batches):
  - e_k is SYMMETRIC in (i,j): the PSUM tile holding e_k[i,j] doubles as
    e_k[j,i], so masking with the host-TRANSPOSED adjacency yields the
    transposed attention weights directly -- no on-chip transposes.
  - All per-oct inputs (hT | adjT | hh) ship as ONE packed DMA; issuing
    a DMA costs ~600ns on the sync engine, so fewer/bigger wins.
  - w_all[d,(l,k,j)] = h[j,d]*a[k,d] is ONE DVE tensor_tensor with
    broadcast reads (hT repeated over k, aPat repeated over l).
  - leaky-relu runs on ACT as Prelu(0.2) evacuating PSUM; Exp follows.
  - Selection is a multiplicative one-hot (is_equal vs a constant k
    pattern); masked entries become exactly 0.  GpSimd is NOT used for
    element-wise work: it shares an SBUF port with the vector engine
    and concurrent ops slow DVE tensor_tensor ~2.4x (measured).
  - A ones-column appended to hidden makes the output matmul emit the
    softmax denominator.  The out PSUM tile puts each pair-block at a
    512-col offset (bank-aligned) so ONE strided ACT copy evacuates
    all four blocks; the division happens on the HOST.
"""

import numpy as np
import ml_dtypes

from contextlib import ExitStack

import concourse.bass as bass
import concourse.tile as tile
from concourse import bacc, mybir
from concourse._compat import with_exitstack
from concourse.bass_utils import run_bass_kernel_spmd

BF16 = mybir.dt.bfloat16
F32 = mybir.dt.float32
ALU = mybir.AluOpType
ACTF = mybir.ActivationFunctionType

B, N, D, K = 512, 64, 128, 4
NCORES = 8
BPC = B // NCORES          # 64 batches per core
OCTS = BPC // 8            # 8 octs of 8 batches per core
HHW = 132                  # hidden cols + ones col + pad (128 data, 1 ones, 3 zero)
OW = 4 * HHW               # output cols (4 pair-blocks of 132)
INW = 512 + 256 + 4 * HHW  # packed input cols: hT | adjT | hh


@with_exitstack
def _kernel_body(ctx, tc, in_d, aPat_d, out_d):
    nc = tc.nc

    const_pool = ctx.enter_context(tc.tile_pool(name="const", bufs=1))
    in_pool = ctx.enter_context(tc.tile_pool(name="inp", bufs=3))
    work_pool = ctx.enter_context(tc.tile_pool(name="work", bufs=2))
    psum_pool = ctx.enter_context(tc.tile_pool(name="psum", bufs=2, space="PSUM"))
    opsum_pool = ctx.enter_context(tc.tile_pool(name="opsum", bufs=1, space="PSUM"))

    # --- one-time constants ---
    # aPat[d, k*64+j] = a[k,d]  (host-precomputed)
    aPat = const_pool.tile([128, 256], BF16)
    nc.sync.dma_start(out=aPat[:], in_=aPat_d[:, :])
    # kpat[:, c*256 + k*64 + s] = k+1  (compare target for the one-hot)
    kpat = const_pool.tile([128, 1024], BF16)
    for c in range(4):
        for k in range(K):
            nc.gpsimd.memset(kpat[:, c * 256 + k * 64 : c * 256 + (k + 1) * 64],
                             float(k + 1))

    for q in range(OCTS):
        # ---- one packed load: hT [d,(l,i)] | adjT [(u,r),(c,s)] | hh ----
        inp = in_pool.tile([128, INW], BF16, tag="inp")
        nc.sync.dma_start(out=inp[:], in_=in_d[q])
        hT = inp[:, 0:512]
        adjT = inp[:, 512:768]
        hh = inp[:, 768:INW]

        # ---- w_all[d, (l,k,j)] = hT[d, (l,j)] * a[k,d] : one DVE op ----
        w_all = work_pool.tile([128, 2048], BF16, tag="w_all")
        w_allv = w_all[:].rearrange("p (l k j) -> p l k j", l=8, k=4)
        hTv = (hT.rearrange("p (l j) -> p l j", l=8)
               .unsqueeze(2).broadcast_to([128, 8, 4, 64]))
        aPatv = (aPat[:].rearrange("p (k j) -> p k j", k=4)
                 .unsqueeze(1).broadcast_to([128, 8, 4, 64]))
        nc.vector.tensor_tensor(w_allv, hTv, aPatv, ALU.mult)

        # ---- e4[(u,i), (c,k,j)] = e_k^{l=2c+u}[i,j] : 8 matmuls ----
        e4 = psum_pool.tile([128, 1024], F32, tag="e4")
        for l in range(8):
            c, u = l // 2, l % 2
            nc.tensor.matmul(
                e4[u * 64 : (u + 1) * 64, c * 256 : (c + 1) * 256],
                lhsT=hT[:, l * 64 : (l + 1) * 64],
                rhs=w_all[:, l * 256 : (l + 1) * 256],
                start=True, stop=True,
                tile_position=(0, u * 64),
            )

        # ---- xm = exp(leakyrelu(e)) : Prelu evacuates PSUM, then Exp ----
        lr4 = work_pool.tile([128, 1024], F32, tag="lr4")
        nc.scalar.activation(lr4[:], e4[:], ACTF.Prelu, alpha=0.2)
        xm = work_pool.tile([128, 1024], BF16, tag="xm")
        nc.scalar.activation(xm[:], lr4[:], ACTF.Exp)

        # ---- one-hot select via transposed adj (symmetry trick) ----
        ind = work_pool.tile([128, 1024], BF16, tag="ind")
        adjv = (adjT.rearrange("p (c s) -> p c s", c=4)
                .unsqueeze(2).broadcast_to([128, 4, 4, 64]))
        kv = kpat[:].rearrange("p (c k s) -> p c k s", c=4, k=4)
        nc.vector.tensor_tensor(
            ind[:].rearrange("p (c k s) -> p c k s", c=4, k=4),
            adjv, kv, ALU.is_equal)
        w4 = work_pool.tile([128, 1024], BF16, tag="w4")
        nc.vector.tensor_mul(w4[:], xm[:], ind[:])

        # ---- sum over k: wsum[(u,j), (c,i)] ----
        w4v = w4[:].rearrange("p (c k s) -> p c k s", c=4, k=4)
        t2 = work_pool.tile([128, 512], BF16, tag="t2")
        t2v = t2[:].rearrange("p (c k s) -> p c k s", c=4, k=2)
        nc.vector.tensor_tensor(t2v, w4v[:, :, 0:2, :], w4v[:, :, 2:4, :], ALU.add)
        wsum = work_pool.tile([128, 256], BF16, tag="wsum")
        wsv = wsum[:].rearrange("p (c s) -> p c s", c=4)
        nc.vector.tensor_tensor(wsv, t2v[:, :, 0, :], t2v[:, :, 1, :], ALU.add)

        # ---- out[(u,i), (c,:)] = sum_j w^T[j,i] hh[j,:]; col 128 = denom ----
        # pair-blocks live at 512-col offsets so each matmul output stays
        # inside one 2 KiB PSUM bank and ONE strided copy evacuates all 4.
        ops = opsum_pool.tile([128, 2048], F32, tag="ops")
        for l in range(8):
            c, u = l // 2, l % 2
            nc.tensor.matmul(
                ops[u * 64 : (u + 1) * 64, c * 512 : c * 512 + HHW],
                lhsT=wsum[u * 64 : (u + 1) * 64, c * 64 : (c + 1) * 64],
                rhs=hh[u * 64 : (u + 1) * 64, c * HHW : (c + 1) * HHW],
                start=True, stop=True,
                tile_position=(u * 64, u * 64),
            )

        # ---- raw numerator+denominator to HBM (host divides) ----
        osb = work_pool.tile([128, OW], F32, tag="osb")
        nc.scalar.activation(
            osb[:].rearrange("p (c w) -> p c w", c=4),
            ops[:].rearrange("p (c z) -> p c z", c=4)[:, :, 0:HHW],
            ACTF.Copy)
        nc.sync.dma_start(out=out_d[q], in_=osb[:])


def build_nc():
    nc = bacc.Bacc("TRN2", target_bir_lowering=False, debug=False)
    in_d = nc.dram_tensor("inp", [OCTS, 128, INW], BF16, kind="ExternalInput").ap()
    aPat_d = nc.dram_tensor("apat", [128, 256], BF16, kind="ExternalInput").ap()
    out_d = nc.dram_tensor("out", [OCTS, 128, OW], F32, kind="ExternalOutput").ap()
    with tile.TileContext(nc) as tc:
        _kernel_body(tc, in_d, aPat_d, out_d)
    nc.compile()
    return nc


def prep_inputs(hidden, adj, a):
    """Host-side packing: bf16 casts, transposed/interleaved layouts, shards."""
    bf = ml_dtypes.bfloat16
    hidden = np.asarray(hidden, dtype=np.float32)
    adj = np.asarray(adj)
    a = np.asarray(a, dtype=np.float32)

    hb = hidden.astype(bf)                                   # [B, 64, 128]

    # hT_q[q, d, l*64+i] = hidden[8q+l, i, d]
    hT = (hb.transpose(0, 2, 1)                              # [B, d, i]
          .reshape(B // 8, 8, D, N)                          # [q, l, d, i]
          .transpose(0, 2, 1, 3)                             # [q, d, l, i]
          .reshape(B // 8, D, 8 * N))

    # adjT_q[q, u*64+r, c*64+s] = adj[8q+2c+u][s, r]
    adjT = adj.transpose(0, 2, 1).astype(bf)                 # [b, r, s]
    adjTq = (adjT.reshape(B // 8, 4, 2, N, N)                # [q, c, u, r, s]
             .transpose(0, 2, 3, 1, 4)                       # [q, u, r, c, s]
             .reshape(B // 8, 2 * N, 4 * N))

    # hh_oct[q][u*64+j, c*132+d] = hidden[8q+2c+u, j, d]; col 128 = 1
    hh = np.zeros((B, N, HHW), dtype=bf)
    hh[:, :, 0:D] = hb
    hh[:, :, D] = bf(1.0)
    hhq = (hh.reshape(B // 8, 4, 2, N, HHW)                  # [q, c, u, j, :]
           .transpose(0, 2, 3, 1, 4)                         # [q, u, j, c, :]
           .reshape(B // 8, 2 * N, 4 * HHW))

    packed = np.concatenate([hT, adjTq, hhq], axis=2)        # [B//8, 128, INW]
    packed = np.ascontiguousarray(packed)

    # aPat[d, k*64+j] = a[k, d]
    aPat = np.ascontiguousarray(
        np.broadcast_to(a.T[:, :, None], (D, K, N)).reshape(D, K * N)
    ).astype(bf)

    in_maps = []
    for cidx in range(NCORES):
        qsl = slice(cidx * OCTS, (cidx + 1) * OCTS)
        in_maps.append({
            "inp": np.ascontiguousarray(packed[qsl]),
            "apat": aPat,
        })
    return in_maps


_NC_CACHE = {}


def run_device(hidden, adj, a, **spmd_kwargs):
    if "nc" not in _NC_CACHE:
        _NC_CACHE["nc"] = build_nc()
    nc = _NC_CACHE["nc"]
    in_maps = prep_inputs(hidden, adj, a)
    res = run_bass_kernel_spmd(nc, in_maps, list(range(NCORES)), **spmd_kwargs)
    raw = np.stack([res.results[c]["out"] for c in range(NCORES)], axis=0)
    # raw: [NCORES, OCTS, 128, 528] f32 -> [b, i, 132] -> normalize
    o = (raw.reshape(NCORES * OCTS, 2, N, 4, HHW)            # [q, u, i, c, :]
         .transpose(0, 3, 1, 2, 4)                           # [q, c, u, i, :]
         .reshape(B, N, HHW))
    out = o[:, :, 0:D] / o[:, :, D:D + 1]
    return np.ascontiguousarray(out, dtype=np.float32), res


def kernel(hidden, adj, a):
    out, _ = run_device(hidden, adj, a)
    return out


# revision 7
# speedup vs baseline: 5.6509x; 1.1005x over previous
"""Bass/Trainium2 kernel for nn_LocalAggregator (GNN message passing).

Math per batch b (hidden [64,128], adj [64,64] in {0..4}, a [4,128]):
    e_k[i,j] = leakyrelu_{0.2}( sum_d hidden[i,d]*hidden[j,d]*a[k,d] )
    alpha    = softmax_j( where(adj==k+1, e_k, -9e15) )
    out      = alpha @ hidden

Device strategy (8 cores, pure batch data-parallel, 64 batches/core,
processed in "octs" of 8 batches):
  - e_k is SYMMETRIC in (i,j): the PSUM tile holding e_k[i,j] doubles as
    e_k[j,i], so masking with the host-TRANSPOSED adjacency yields the
    transposed attention weights directly -- no on-chip transposes.
  - All per-oct inputs (hT | adjT | hh) ship as ONE packed DMA; issuing
    a DMA costs ~600ns on the sync engine, so fewer/bigger wins.
  - w_all[d,(l,k,j)] = h[j,d]*a[k,d] is ONE DVE tensor_tensor with
    broadcast reads (hT repeated over k, aPat repeated over l).
  - leaky-relu runs on ACT as Prelu(0.2) evacuating PSUM; Exp follows.
  - Selection is a multiplicative one-hot (is_equal vs a constant k
    pattern); masked entries become exactly 0.  GpSimd is NOT used for
    element-wise work: it shares an SBUF port with the vector engine
    and concurrent ops slow DVE tensor_tensor ~2.4x (measured).
  - A ones-column appended to hidden makes the output matmul emit the
    softmax denominator.  The out PSUM tile puts each pair-block at a
    512-col offset (bank-aligned) so ONE strided ACT copy evacuates
    all four blocks; the division happens on the HOST.
"""

import numpy as np
import ml_dtypes

from contextlib import ExitStack

import concourse.bass as bass
import concourse.tile as tile
from concourse import bacc, mybir
from concourse._compat import with_exitstack
from concourse.bass_utils import run_bass_kernel_spmd

BF16 = mybir.dt.bfloat16
F32 = mybir.dt.float32
ALU = mybir.AluOpType
ACTF = mybir.ActivationFunctionType

B, N, D, K = 512, 64, 128, 4
NCORES = 8
BPC = B // NCORES          # 64 batches per core
OCTS = BPC // 8            # 8 octs of 8 batches per core
HHW = 132                  # hidden cols + ones col + pad (128 data, 1 ones, 3 zero)
OW = 4 * HHW               # output cols (4 pair-blocks of 132)
INW = 512 + 256 + 4 * HHW  # packed input cols: hT | adjT | hh


@with_exitstack
def _kernel_body(ctx, tc, in_d, aPat_d, out_d):
    nc = tc.nc

    const_pool = ctx.enter_context(tc.tile_pool(name="const", bufs=1))
    in_pool = ctx.enter_context(tc.tile_pool(name="inp", bufs=4))
    work_pool = ctx.enter_context(tc.tile_pool(name="work", bufs=4))
    psum_pool = ctx.enter_context(tc.tile_pool(name="psum", bufs=2, space="PSUM"))
    opsum_pool = ctx.enter_context(tc.tile_pool(name="opsum", bufs=1, space="PSUM"))

    # --- one-time constants ---
    # aPat[d, k*64+j] = a[k,d]  (host-precomputed)
    aPat = const_pool.tile([128, 256], BF16)
    nc.sync.dma_start(out=aPat[:], in_=aPat_d[:, :])
    # kpat[:, c*256 + k*64 + s] = k+1  (compare target for the one-hot)
    kpat = const_pool.tile([128, 1024], BF16)
    for c in range(4):
        for k in range(K):
            nc.gpsimd.memset(kpat[:, c * 256 + k * 64 : c * 256 + (k + 1) * 64],
                             float(k + 1))

    for q in range(OCTS):
        # ---- one packed load: hT [d,(l,i)] | adjT [(u,r),(c,s)] | hh ----
        inp = in_pool.tile([128, INW], BF16, tag="inp")
        nc.sync.dma_start(out=inp[:], in_=in_d[q])
        hT = inp[:, 0:512]
        adjT = inp[:, 512:768]
        hh = inp[:, 768:INW]

        # ---- w_all[d, (l,k,j)] = hT[d, (l,j)] * a[k,d] : one DVE op ----
        w_all = work_pool.tile([128, 2048], BF16, tag="w_all")
        w_allv = w_all[:].rearrange("p (l k j) -> p l k j", l=8, k=4)
        hTv = (hT.rearrange("p (l j) -> p l j", l=8)
               .unsqueeze(2).broadcast_to([128, 8, 4, 64]))
        aPatv = (aPat[:].rearrange("p (k j) -> p k j", k=4)
                 .unsqueeze(1).broadcast_to([128, 8, 4, 64]))
        nc.vector.tensor_tensor(w_allv, hTv, aPatv, ALU.mult)

        # ---- e4[(u,i), (c,k,j)] = e_k^{l=2c+u}[i,j] : 8 matmuls ----
        e4 = psum_pool.tile([128, 1024], F32, tag="e4")
        for l in range(8):
            c, u = l // 2, l % 2
            nc.tensor.matmul(
                e4[u * 64 : (u + 1) * 64, c * 256 : (c + 1) * 256],
                lhsT=hT[:, l * 64 : (l + 1) * 64],
                rhs=w_all[:, l * 256 : (l + 1) * 256],
                start=True, stop=True,
                tile_position=(0, u * 64),
            )

        # ---- xm = exp(leakyrelu(e)) : Prelu evacuates PSUM, then Exp ----
        lr4 = work_pool.tile([128, 1024], F32, tag="lr4")
        nc.scalar.activation(lr4[:], e4[:], ACTF.Prelu, alpha=0.2)
        xm = work_pool.tile([128, 1024], BF16, tag="xm")
        nc.scalar.activation(xm[:], lr4[:], ACTF.Exp)

        # ---- one-hot select via transposed adj (symmetry trick) ----
        ind = work_pool.tile([128, 1024], BF16, tag="ind")
        adjv = (adjT.rearrange("p (c s) -> p c s", c=4)
                .unsqueeze(2).broadcast_to([128, 4, 4, 64]))
        kv = kpat[:].rearrange("p (c k s) -> p c k s", c=4, k=4)
        nc.vector.tensor_tensor(
            ind[:].rearrange("p (c k s) -> p c k s", c=4, k=4),
            adjv, kv, ALU.is_equal)
        w4 = work_pool.tile([128, 1024], BF16, tag="w4")
        nc.vector.tensor_mul(w4[:], xm[:], ind[:])

        # ---- sum over k: wsum[(u,j), (c,i)] ----
        w4v = w4[:].rearrange("p (c k s) -> p c k s", c=4, k=4)
        t2 = work_pool.tile([128, 512], BF16, tag="t2")
        t2v = t2[:].rearrange("p (c k s) -> p c k s", c=4, k=2)
        nc.vector.tensor_tensor(t2v, w4v[:, :, 0:2, :], w4v[:, :, 2:4, :], ALU.add)
        wsum = work_pool.tile([128, 256], BF16, tag="wsum")
        wsv = wsum[:].rearrange("p (c s) -> p c s", c=4)
        nc.vector.tensor_tensor(wsv, t2v[:, :, 0, :], t2v[:, :, 1, :], ALU.add)

        # ---- out[(u,i), (c,:)] = sum_j w^T[j,i] hh[j,:]; col 128 = denom ----
        # pair-blocks live at 512-col offsets so each matmul output stays
        # inside one 2 KiB PSUM bank and ONE strided copy evacuates all 4.
        ops = opsum_pool.tile([128, 2048], F32, tag="ops")
        for l in range(8):
            c, u = l // 2, l % 2
            nc.tensor.matmul(
                ops[u * 64 : (u + 1) * 64, c * 512 : c * 512 + HHW],
                lhsT=wsum[u * 64 : (u + 1) * 64, c * 64 : (c + 1) * 64],
                rhs=hh[u * 64 : (u + 1) * 64, c * HHW : (c + 1) * HHW],
                start=True, stop=True,
                tile_position=(u * 64, u * 64),
            )

        # ---- raw numerator+denominator to HBM (host divides) ----
        osb = work_pool.tile([128, OW], F32, tag="osb")
        nc.scalar.activation(
            osb[:].rearrange("p (c w) -> p c w", c=4),
            ops[:].rearrange("p (c z) -> p c z", c=4)[:, :, 0:HHW],
            ACTF.Copy)
        nc.sync.dma_start(out=out_d[q], in_=osb[:])


def build_nc():
    nc = bacc.Bacc("TRN2", target_bir_lowering=False, debug=False)
    in_d = nc.dram_tensor("inp", [OCTS, 128, INW], BF16, kind="ExternalInput").ap()
    aPat_d = nc.dram_tensor("apat", [128, 256], BF16, kind="ExternalInput").ap()
    out_d = nc.dram_tensor("out", [OCTS, 128, OW], F32, kind="ExternalOutput").ap()
    with tile.TileContext(nc) as tc:
        _kernel_body(tc, in_d, aPat_d, out_d)
    nc.compile()
    return nc


def prep_inputs(hidden, adj, a):
    """Host-side packing: bf16 casts, transposed/interleaved layouts, shards."""
    bf = ml_dtypes.bfloat16
    hidden = np.asarray(hidden, dtype=np.float32)
    adj = np.asarray(adj)
    a = np.asarray(a, dtype=np.float32)

    hb = hidden.astype(bf)                                   # [B, 64, 128]

    # hT_q[q, d, l*64+i] = hidden[8q+l, i, d]
    hT = (hb.transpose(0, 2, 1)                              # [B, d, i]
          .reshape(B // 8, 8, D, N)                          # [q, l, d, i]
          .transpose(0, 2, 1, 3)                             # [q, d, l, i]
          .reshape(B // 8, D, 8 * N))

    # adjT_q[q, u*64+r, c*64+s] = adj[8q+2c+u][s, r]
    adjT = adj.transpose(0, 2, 1).astype(bf)                 # [b, r, s]
    adjTq = (adjT.reshape(B // 8, 4, 2, N, N)                # [q, c, u, r, s]
             .transpose(0, 2, 3, 1, 4)                       # [q, u, r, c, s]
             .reshape(B // 8, 2 * N, 4 * N))

    # hh_oct[q][u*64+j, c*132+d] = hidden[8q+2c+u, j, d]; col 128 = 1
    hh = np.zeros((B, N, HHW), dtype=bf)
    hh[:, :, 0:D] = hb
    hh[:, :, D] = bf(1.0)
    hhq = (hh.reshape(B // 8, 4, 2, N, HHW)                  # [q, c, u, j, :]
           .transpose(0, 2, 3, 1, 4)                         # [q, u, j, c, :]
           .reshape(B // 8, 2 * N, 4 * HHW))

    packed = np.concatenate([hT, adjTq, hhq], axis=2)        # [B//8, 128, INW]
    packed = np.ascontiguousarray(packed)

    # aPat[d, k*64+j] = a[k, d]
    aPat = np.ascontiguousarray(
        np.broadcast_to(a.T[:, :, None], (D, K, N)).reshape(D, K * N)
    ).astype(bf)

    in_maps = []
    for cidx in range(NCORES):
        qsl = slice(cidx * OCTS, (cidx + 1) * OCTS)
        in_maps.append({
            "inp": np.ascontiguousarray(packed[qsl]),
            "apat": aPat,
        })
    return in_maps


_NC_CACHE = {}


def run_device(hidden, adj, a, **spmd_kwargs):
    if "nc" not in _NC_CACHE:
        _NC_CACHE["nc"] = build_nc()
    nc = _NC_CACHE["nc"]
    in_maps = prep_inputs(hidden, adj, a)
    res = run_bass_kernel_spmd(nc, in_maps, list(range(NCORES)), **spmd_kwargs)
    raw = np.stack([res.results[c]["out"] for c in range(NCORES)], axis=0)
    # raw: [NCORES, OCTS, 128, 528] f32 -> [b, i, 132] -> normalize
    o = (raw.reshape(NCORES * OCTS, 2, N, 4, HHW)            # [q, u, i, c, :]
         .transpose(0, 3, 1, 2, 4)                           # [q, c, u, i, :]
         .reshape(B, N, HHW))
    out = o[:, :, 0:D] / o[:, :, D:D + 1]
    return np.ascontiguousarray(out, dtype=np.float32), res


def kernel(hidden, adj, a):
    out, _ = run_device(hidden, adj, a)
    return out


# revision 10
# speedup vs baseline: 5.9579x; 1.0543x over previous
"""Bass/Trainium2 kernel for nn_LocalAggregator (GNN message passing).

Math per batch b (hidden [64,128], adj [64,64] in {0..4}, a [4,128]):
    e_k[i,j] = leakyrelu_{0.2}( sum_d hidden[i,d]*hidden[j,d]*a[k,d] )
    alpha    = softmax_j( where(adj==k+1, e_k, -9e15) )
    out      = alpha @ hidden

Device strategy (8 cores, pure batch data-parallel, 64 batches/core,
processed in "octs" of 8 batches):
  - e_k is SYMMETRIC in (i,j): the PSUM tile holding e_k[i,j] doubles as
    e_k[j,i], so masking with the host-TRANSPOSED adjacency yields the
    transposed attention weights directly -- no on-chip transposes.
  - All per-oct inputs (hT | adjT | hh) ship as ONE packed DMA; issuing
    a DMA costs ~600ns on the sync engine, so fewer/bigger wins.
  - w_all[d,(l,k,j)] = h[j,d]*a[k,d] is ONE DVE tensor_tensor with
    broadcast reads (hT repeated over k, aPat repeated over l).
  - leaky-relu runs on ACT as Prelu(0.2) evacuating PSUM; Exp follows.
  - Selection is a multiplicative one-hot (is_equal vs a constant k
    pattern); masked entries become exactly 0.  GpSimd is NOT used for
    element-wise work: it shares an SBUF port with the vector engine
    and concurrent ops slow DVE tensor_tensor ~2.4x (measured).
  - A ones-column appended to hidden makes the output matmul emit the
    softmax denominator.  The out PSUM tile puts each pair-block at a
    512-col offset (bank-aligned) so ONE strided ACT copy evacuates
    all four blocks; the division happens on the HOST.
"""

import numpy as np
import ml_dtypes

from contextlib import ExitStack

import concourse.bass as bass
import concourse.tile as tile
from concourse import bacc, mybir
from concourse._compat import with_exitstack
from concourse.bass_utils import run_bass_kernel_spmd

BF16 = mybir.dt.bfloat16
F32 = mybir.dt.float32
ALU = mybir.AluOpType
ACTF = mybir.ActivationFunctionType

B, N, D, K = 512, 64, 128, 4
NCORES = 8
BPC = B // NCORES          # 64 batches per core
OCTS = BPC // 8            # 8 octs of 8 batches per core
HHW = 132                  # hidden cols + ones col + pad (128 data, 1 ones, 3 zero)
OW = 4 * HHW               # output cols (4 pair-blocks of 132)
INW = 512 + 256 + 4 * HHW  # packed input cols: hT | adjT | hh


@with_exitstack
def _kernel_body(ctx, tc, in_d, aPat_d, out_d):
    nc = tc.nc

    const_pool = ctx.enter_context(tc.tile_pool(name="const", bufs=1))
    in_pool = ctx.enter_context(tc.tile_pool(name="inp", bufs=4))
    work_pool = ctx.enter_context(tc.tile_pool(name="work", bufs=4))
    psum_pool = ctx.enter_context(tc.tile_pool(name="psum", bufs=2, space="PSUM"))
    opsum_pool = ctx.enter_context(tc.tile_pool(name="opsum", bufs=2, space="PSUM"))

    # --- one-time constants ---
    # aPat[d, k*64+j] = a[k,d]  (host-precomputed)
    aPat = const_pool.tile([128, 256], BF16)
    nc.sync.dma_start(out=aPat[:], in_=aPat_d[:, :])
    # kpat[:, c*256 + k*64 + s] = k+1  (compare target for the one-hot)
    kpat = const_pool.tile([128, 1024], BF16)
    for c in range(4):
        for k in range(K):
            nc.gpsimd.memset(kpat[:, c * 256 + k * 64 : c * 256 + (k + 1) * 64],
                             float(k + 1))

    for q in range(OCTS):
        # ---- packed load, split so the critical hT|adjT part lands first
        inp = in_pool.tile([128, INW], BF16, tag="inp")
        nc.sync.dma_start(out=inp[:, 0:768], in_=in_d[q][:, 0:768])
        nc.sync.dma_start(out=inp[:, 768:INW], in_=in_d[q][:, 768:INW])
        hT = inp[:, 0:512]
        adjT = inp[:, 512:768]
        hh = inp[:, 768:INW]

        # ---- w_all[d, (l,k,j)] = hT[d, (l,j)] * a[k,d] : one DVE op ----
        w_all = work_pool.tile([128, 2048], BF16, tag="w_all")
        w_allv = w_all[:].rearrange("p (l k j) -> p l k j", l=8, k=4)
        hTv = (hT.rearrange("p (l j) -> p l j", l=8)
               .unsqueeze(2).broadcast_to([128, 8, 4, 64]))
        aPatv = (aPat[:].rearrange("p (k j) -> p k j", k=4)
                 .unsqueeze(1).broadcast_to([128, 8, 4, 64]))
        nc.vector.tensor_tensor(w_allv, hTv, aPatv, ALU.mult)

        # ---- e4[(u,i), (c,k,j)] = e_k^{l=2c+u}[i,j] : 8 matmuls ----
        e4 = psum_pool.tile([128, 1024], F32, tag="e4")
        for l in range(8):
            c, u = l // 2, l % 2
            nc.tensor.matmul(
                e4[u * 64 : (u + 1) * 64, c * 256 : (c + 1) * 256],
                lhsT=hT[:, l * 64 : (l + 1) * 64],
                rhs=w_all[:, l * 256 : (l + 1) * 256],
                start=True, stop=True,
                tile_position=(0, u * 64),
            )

        # ---- xm = exp(leakyrelu(e)) : Prelu evacuates PSUM, then Exp ----
        lr4 = work_pool.tile([128, 1024], F32, tag="lr4")
        nc.scalar.activation(lr4[:], e4[:], ACTF.Prelu, alpha=0.2)
        xm = work_pool.tile([128, 1024], BF16, tag="xm")
        nc.scalar.activation(xm[:], lr4[:], ACTF.Exp)

        # ---- one-hot select via transposed adj (symmetry trick) ----
        ind = work_pool.tile([128, 1024], BF16, tag="ind")
        adjv = (adjT.rearrange("p (c s) -> p c s", c=4)
                .unsqueeze(2).broadcast_to([128, 4, 4, 64]))
        kv = kpat[:].rearrange("p (c k s) -> p c k s", c=4, k=4)
        nc.vector.tensor_tensor(
            ind[:].rearrange("p (c k s) -> p c k s", c=4, k=4),
            adjv, kv, ALU.is_equal)
        w4 = work_pool.tile([128, 1024], BF16, tag="w4")
        nc.vector.tensor_mul(w4[:], xm[:], ind[:])

        # ---- partial sum over k: t2[(u,j), (c,k2,i)] ----
        w4v = w4[:].rearrange("p (c k s) -> p c k s", c=4, k=4)
        t2 = work_pool.tile([128, 512], BF16, tag="t2")
        t2v = t2[:].rearrange("p (c k s) -> p c k s", c=4, k=2)
        nc.vector.tensor_tensor(t2v, w4v[:, :, 0:2, :], w4v[:, :, 2:4, :], ALU.add)

        # ---- out[(u,i), (c,:)] = sum_j w^T[j,i] hh[j,:]; col 128 = denom ----
        # the remaining k-pair sum rides on PSUM accumulation (2 matmuls).
        # pair-blocks live at 256-col offsets so no matmul output crosses
        # a 2 KiB PSUM bank and ONE strided copy evacuates all 4.
        ops = opsum_pool.tile([128, 1024], F32, tag="ops")
        for l in range(8):
            c, u = l // 2, l % 2
            for h in range(2):
                nc.tensor.matmul(
                    ops[u * 64 : (u + 1) * 64, c * 256 : c * 256 + HHW],
                    lhsT=t2[u * 64 : (u + 1) * 64,
                            c * 128 + h * 64 : c * 128 + (h + 1) * 64],
                    rhs=hh[u * 64 : (u + 1) * 64, c * HHW : (c + 1) * HHW],
                    start=(h == 0), stop=(h == 1),
                    tile_position=(u * 64, u * 64),
                )

        # ---- raw numerator+denominator to HBM (host divides) ----
        osb = work_pool.tile([128, OW], F32, tag="osb")
        nc.scalar.activation(
            osb[:].rearrange("p (c w) -> p c w", c=4),
            ops[:].rearrange("p (c z) -> p c z", c=4)[:, :, 0:HHW],
            ACTF.Copy)
        nc.sync.dma_start(out=out_d[q], in_=osb[:])


def build_nc():
    nc = bacc.Bacc("TRN2", target_bir_lowering=False, debug=False)
    in_d = nc.dram_tensor("inp", [OCTS, 128, INW], BF16, kind="ExternalInput").ap()
    aPat_d = nc.dram_tensor("apat", [128, 256], BF16, kind="ExternalInput").ap()
    out_d = nc.dram_tensor("out", [OCTS, 128, OW], F32, kind="ExternalOutput").ap()
    with tile.TileContext(nc) as tc:
        _kernel_body(tc, in_d, aPat_d, out_d)
    nc.compile()
    return nc


def prep_inputs(hidden, adj, a):
    """Host-side packing: bf16 casts, transposed/interleaved layouts, shards."""
    bf = ml_dtypes.bfloat16
    hidden = np.asarray(hidden, dtype=np.float32)
    adj = np.asarray(adj)
    a = np.asarray(a, dtype=np.float32)

    hb = hidden.astype(bf)                                   # [B, 64, 128]

    # hT_q[q, d, l*64+i] = hidden[8q+l, i, d]
    hT = (hb.transpose(0, 2, 1)                              # [B, d, i]
          .reshape(B // 8, 8, D, N)                          # [q, l, d, i]
          .transpose(0, 2, 1, 3)                             # [q, d, l, i]
          .reshape(B // 8, D, 8 * N))

    # adjT_q[q, u*64+r, c*64+s] = adj[8q+2c+u][s, r]
    adjT = adj.transpose(0, 2, 1).astype(bf)                 # [b, r, s]
    adjTq = (adjT.reshape(B // 8, 4, 2, N, N)                # [q, c, u, r, s]
             .transpose(0, 2, 3, 1, 4)                       # [q, u, r, c, s]
             .reshape(B // 8, 2 * N, 4 * N))

    # hh_oct[q][u*64+j, c*132+d] = hidden[8q+2c+u, j, d]; col 128 = 1
    hh = np.zeros((B, N, HHW), dtype=bf)
    hh[:, :, 0:D] = hb
    hh[:, :, D] = bf(1.0)
    hhq = (hh.reshape(B // 8, 4, 2, N, HHW)                  # [q, c, u, j, :]
           .transpose(0, 2, 3, 1, 4)                         # [q, u, j, c, :]
           .reshape(B // 8, 2 * N, 4 * HHW))

    packed = np.concatenate([hT, adjTq, hhq], axis=2)        # [B//8, 128, INW]
    packed = np.ascontiguousarray(packed)

    # aPat[d, k*64+j] = a[k, d]
    aPat = np.ascontiguousarray(
        np.broadcast_to(a.T[:, :, None], (D, K, N)).reshape(D, K * N)
    ).astype(bf)

    in_maps = []
    for cidx in range(NCORES):
        qsl = slice(cidx * OCTS, (cidx + 1) * OCTS)
        in_maps.append({
            "inp": np.ascontiguousarray(packed[qsl]),
            "apat": aPat,
        })
    return in_maps


_NC_CACHE = {}


def run_device(hidden, adj, a, **spmd_kwargs):
    if "nc" not in _NC_CACHE:
        _NC_CACHE["nc"] = build_nc()
    nc = _NC_CACHE["nc"]
    in_maps = prep_inputs(hidden, adj, a)
    res = run_bass_kernel_spmd(nc, in_maps, list(range(NCORES)), **spmd_kwargs)
    raw = np.stack([res.results[c]["out"] for c in range(NCORES)], axis=0)
    # raw: [NCORES, OCTS, 128, 528] f32 -> [b, i, 132] -> normalize
    o = (raw.reshape(NCORES * OCTS, 2, N, 4, HHW)            # [q, u, i, c, :]
         .transpose(0, 3, 1, 2, 4)                           # [q, c, u, i, :]
         .reshape(B, N, HHW))
    out = o[:, :, 0:D] / o[:, :, D:D + 1]
    return np.ascontiguousarray(out, dtype=np.float32), res


def kernel(hidden, adj, a):
    out, _ = run_device(hidden, adj, a)
    return out
